# revision 8
# baseline (speedup 1.0000x reference)
"""Trainium2 Bass kernel for nn_ConceptEmbedding (type-conditioned embedding lookup).

v27 strategy (= v21 balanced overlapping windows + v26 fused parallel
half-stores + v18 no-final-wait; builds on the v16 baseline's SWDGE
behavioral notes measured on HW over v3-v15):
  - Fold the three tables into one [3V, E] bf16 table; row-range shard across
    8 cores (37632-row slice per core, host-staged as "twin").
  - OVERLAPPING WINDOWS: the int16 gather index caps a window at 32768 rows,
    but windows may overlap. Window A = twin[0:32768], window B =
    twin[4864:37632]. Rows in the 27.9K-row overlap are assignable to either
    window, so the host balances both sides to ~1536 typed tokens each
    (cap 1664 apiece). This removes the unbalanced 4864-row remainder window,
    equalizing gather sizes to (896, 896, 768, 768) instead of
    (1024, 896, 896, 512): G0's synchronous descriptor-gen (engine-blocking
    at ~5.9ns/idx of STATIC cap) drops by ~0.76us. (A strided elem_step
    parity scheme would do the same but walrus' visitInstDMAGatherAnt
    rejects non-contiguous gather sources.)
  - Warm-up (32 idx) kept: it eats the mandatory burst-head sync (~1us +
    ~9ns/idx); G0 then size-syncs at the cheaper ~5.9ns/idx with its drain
    streaming (single_packet=False), G1-G3 defer (~4ns/idx on their own Q7
    queue pairs). Queues rotate 0(warm),1,2,3,0 - the q0 reuse lands ~8us
    after the warm-up, whose 8KB drain completes ~2.5us in (baseline-proven).
  - TWO fused half-stores (sync + scalar), each issued once after ALL four
    gather completion sems, with NO final DMA-completion wait: the block-exit
    drains + NEFF epilogue fence the store DMAs before the host reads `out`
    (v18-verified on HW), so store execution hides under the fixed ~7.4us
    teardown and the drain phase runs free of store competition.
  - Host: window-balances typed tokens per core (sorted ascending for
    ascending HBM addresses), pads to caps, gathers statistical overflow on
    host, unpermutes while assembling the output.

dma_gather layout facts (verified on HW):
  - indices live at idxs[i % 16, i // 16], int16, replicated across all 128
    partitions; valid index i lands at dst[i % 128, i // 128, :].
  - the store view maps SBUF (p, block b) -> DRAM row p*NB + b, so the DRAM
    row for slot j of a region at block base B0 is (j % 128)*NB + B0 + j//128.
"""

import numpy as np

V = 100000
E = 128
B = 16
S = 2048
NCORES = 8
P = 128

N_TOK = B * S  # 32768
NROWS = 3 * V  # 300000

RSPAN = 37504  # table rows owned per core (8 * 37504 >= 300000)
TWLEN = 37632  # per-core table slice length (RSPAN + 128 alignment margin)
WND = 32768  # rows addressable per window (int16 index range)
B0ROW = TWLEN - WND  # 4864: window B covers twin[B0ROW:TWLEN]

# Per-window slot caps. Typed tokens per core ~3072 (std ~52); host-balanced
# ~1536 per window (cap 1664 apiece, overflow tail gathered on host).
PCAP = 1664
SUMCAP = 2 * PCAP  # 3328
NB = SUMCAP // P  # 26 blocks
WB0 = PCAP  # first slot of the window-B region

# Device issue order: (window, slot0, cap, queue). Sizes non-increasing after
# the warm-up head so G1-G3 defer; equalized to minimize G0's engine-blocking
# sync-gen (prop. to its STATIC cap) and the deferred-gen tail.
GATHERS = [
    (0, 0, 896, 1),
    (1, WB0, 896, 2),
    (0, 896, 768, 3),
    (1, WB0 + 896, 768, 0),
]
WARM = 32
WARMREAL = 32
IDXCOLS = SUMCAP // 16 + WARM // 16  # 208 data + 2 warm-up columns

_CACHED_NC = None


def _build_bass():
    global _CACHED_NC
    if _CACHED_NC is not None:
        return _CACHED_NC

    import concourse.bacc as bacc
    import concourse.mybir as mybir
    from concourse.library_config import mlp

    nc = bacc.Bacc(num_swdge_queues=4, dynamic_dma_scratch_size=65536)
    twin = nc.dram_tensor("twin", [TWLEN, E], mybir.dt.bfloat16, kind="ExternalInput")
    idx = nc.dram_tensor("idx", [P, IDXCOLS], mybir.dt.int16, kind="ExternalInput")
    out = nc.dram_tensor("out", [SUMCAP, E], mybir.dt.bfloat16, kind="ExternalOutput")

    # SBUF (p, block b) <-> DRAM row p*NB + b
    out_v = out.rearrange("(p b) e -> p (b e)", p=P)
    # Overlapping gather windows, each 32768 rows (int16-addressable).
    wviews = [twin[0:WND, :], twin[B0ROW:TWLEN, :]]

    with (
        nc.Block(no_gpsimd_drain=True) as block,
        nc.sbuf_tensor("dst", [P, NB * E], mybir.dt.bfloat16) as dst,
        nc.sbuf_tensor("idxs", [P, IDXCOLS], mybir.dt.int16) as idxs,
        nc.sbuf_tensor("wdst", [P, E], mybir.dt.bfloat16) as wdst,
        nc.semaphore("io") as io,
        nc.semaphore("s0") as s0,
        nc.semaphore("s1") as s1,
        nc.semaphore("s2") as s2,
        nc.semaphore("s3") as s3,
    ):
        ssems = [s0, s1, s2, s3]

        # v26-style fused parallel half-stores: store EXECUTION rides into
        # the teardown (no final wait), so only store ISSUE + barrier gate
        # the block end; one half-store per engine after ALL gather sems
        # also keeps the drain phase free of store competition. The inc is
        # mandatory (walrus asserts every DMA has an update sem) but nothing
        # waits on io past 16.
        HALF = (NB // 2) * E

        def _fused_store(eng, lo, hi):
            for ss in ssems:
                eng.wait_ge(ss, 16)
            eng.dma_start(out=out_v[:, lo:hi], in_=dst[:, lo:hi]).then_inc(io, 16)

        @block.gpsimd
        def _(gpsimd):
            gpsimd.load_library(mlp)
            gpsimd.wait_ge(io, 16)
            wd3 = wdst[:, :].rearrange("p (b e) -> p b e", e=E)
            gpsimd.dma_gather(
                wd3,
                wviews[0],
                idxs[:, SUMCAP // 16 : IDXCOLS],
                WARM,
                WARM,
                E,
                queue_num=0,
                # A completion sem is mandatory: the Q7 generator always
                # appends a sem-inc descriptor and walrus' codegen rejects a
                # gather without one. Nothing waits on io past 16.
            ).then_inc(io, 16)
            # Single back-to-back burst after the warm-up head; G0 size-syncs
            # on the engine with its drain streaming, G1-G3 defer to their Q7
            # queue pairs.
            for k, (w, slot0, cap, qn) in enumerate(GATHERS):
                b0 = slot0 // P
                d3 = dst[:, b0 * E : (b0 + cap // P) * E].rearrange(
                    "p (b e) -> p b e", e=E
                )
                gpsimd.dma_gather(
                    d3,
                    wviews[w],
                    idxs[:, slot0 // 16 : (slot0 + cap) // 16],
                    cap,
                    cap,
                    E,
                    queue_num=qn,
                    single_packet=False,
                ).then_inc(ssems[k], 16)
        @block.sync
        def _(sync):
            sync.dma_start(out=idxs[:], in_=idx[:]).then_inc(io, 16)
            _fused_store(sync, 0, HALF)

        @block.scalar
        def _(scalar):
            _fused_store(scalar, HALF, NB * E)

    nc.finalize()
    _CACHED_NC = nc
    return nc


def _shard_inputs(proc_emb, med_emb, chart_emb, concept, token_type):
    """Returns (in_maps, plans, tables) with per-core slot bookkeeping."""
    import ml_dtypes

    tables = np.ascontiguousarray(
        np.concatenate(
            [
                np.asarray(proc_emb, dtype=np.float32),
                np.asarray(med_emb, dtype=np.float32),
                np.asarray(chart_emb, dtype=np.float32),
            ],
            axis=0,
        )
    )
    tables16 = tables.astype(ml_dtypes.bfloat16)
    tt = np.asarray(token_type).reshape(-1).astype(np.int64)
    cc = np.asarray(concept).reshape(-1).astype(np.int64)
    typed = (tt >= 1) & (tt <= 3)
    toks_all = np.where(typed)[0]  # global token ids with a real lookup
    eff = cc[toks_all] + (tt[toks_all] - 1) * V  # their table rows

    core_of = eff // RSPAN
    local = eff - core_of * RSPAN

    # Warm-up indices: WARMREAL distinct, spread window-0 rows, then
    # trailing -1s (truncated by the Q7 descriptor generator).
    warm = np.full(WARM, -1, dtype=np.int16)
    warm[:WARMREAL] = np.arange(WARMREAL, dtype=np.int16) * 977
    warm16 = warm.reshape(WARM // 16, 16).T

    in_maps = []
    plans = []  # per core: (tokens, dram_rows, overflow_tokens, overflow_rows)
    for c in range(NCORES):
        base = c * RSPAN
        sl = tables16[base : min(base + TWLEN, NROWS)]
        if sl.shape[0] < TWLEN:
            sl = np.concatenate(
                [sl, np.zeros((TWLEN - sl.shape[0], E), ml_dtypes.bfloat16)]
            )
        twin = np.ascontiguousarray(sl)

        sel = np.where(core_of == c)[0]
        order = sel[np.argsort(local[sel], kind="stable")]
        lrows = local[order]  # ascending
        ltoks = toks_all[order]
        # Window assignment: rows < B0ROW must use window A, rows >= WND must
        # use window B; the overlap region splits to balance both sides to
        # ~total/2 (ascending order preserved on both sides).
        n_low = int(np.searchsorted(lrows, B0ROW))  # forced into A
        n_high = len(lrows) - int(np.searchsorted(lrows, WND))  # forced into B
        n_mid = len(lrows) - n_low - n_high
        na = min(max((len(lrows) + 1) // 2 - n_low, 0), n_mid) + n_low
        win_lists = [
            (lrows[:na], ltoks[:na], 0, 0),
            (lrows[na:] - B0ROW, ltoks[na:], WB0, WB0 // P),
        ]

        idx16 = np.zeros((16, IDXCOLS), dtype=np.int16)
        idx16[:, SUMCAP // 16 :] = warm16
        tok_list, row_list, ovf_toks, ovf_rows = [], [], [], []
        for wrows, wtoks, slot0, b0 in win_lists:
            cnt = len(wrows)
            if cnt > PCAP:
                # Statistical-tail safety valve: gather the overflow on host.
                ovf_toks.extend(wtoks[PCAP:].tolist())
                ovf_rows.extend((wrows[PCAP:] + (0 if slot0 == 0 else B0ROW)).tolist())
                wrows, wtoks, cnt = wrows[:PCAP], wtoks[:PCAP], PCAP
            vals = np.full(PCAP, -1, dtype=np.int16)  # trailing -1 pads skipped
            vals[:cnt] = wrows.astype(np.int16)
            idx16[:, slot0 // 16 : (slot0 + PCAP) // 16] = vals.reshape(
                PCAP // 16, 16
            ).T
            j = np.arange(cnt)
            row_list.append((j % P) * NB + b0 + j // P)
            tok_list.append(wtoks)

        in_maps.append(
            {"twin": twin, "idx": np.ascontiguousarray(np.tile(idx16, (8, 1)))}
        )
        plans.append(
            (
                np.concatenate(tok_list),
                np.concatenate(row_list),
                np.array(ovf_toks, dtype=np.int64),
                np.array(ovf_rows, dtype=np.int64) + base,
            )
        )

    return in_maps, plans, tables


def _run(in_maps, trace=False):
    from concourse.bass_utils import run_bass_kernel_spmd

    nc = _build_bass()
    return run_bass_kernel_spmd(nc, in_maps, list(range(NCORES)), trace=trace)


def _assemble(results, plans, tables):
    out = np.zeros((N_TOK, E), dtype=np.float32)
    for c in range(NCORES):
        toks, drows, ovf_toks, ovf_rows = plans[c]
        if len(toks):
            out[toks] = results[c]["out"][drows].astype(np.float32)
        if len(ovf_toks):
            out[ovf_toks] = tables[ovf_rows]
    return out.reshape(B, S, E)


def kernel(proc_emb, med_emb, chart_emb, concept, token_type):
    in_maps, plans, tables = _shard_inputs(
        proc_emb, med_emb, chart_emb, concept, token_type
    )
    res = _run(in_maps, trace=False)
    return _assemble(res.results, plans, tables)
